# revision 6
# baseline (speedup 1.0000x reference)
"""Trainium2 Bass kernel for nn_ExperimentalPathwayModel (resonant spiking RNN).

Strategy (data-parallel over batch, 8 cores x 256 rows, H-major layout
[128 h-partitions x 256 batch free] per tile):

  - The affine state shift m = m^ + p, r = r^ + q (p = base(1-d)/((1-d)^2+f),
    q = f p/(1-d)) eliminates the per-step "+ base" drive from the loop.
    With na = -d*r^ the per-step math is
        M    = (d-f) m^ + na + rec        (rec = s_{t-1} @ W_rec)
        s    = 1[M > Theta],  Theta = 1 - p
        m^'  = M - s
        na'  = d na - (d f) m^
  - W_rec matmul runs on the PE in float32r (1 cyc/row) as a 2-pass
    residual split W = R1 + R2 (R1 = 10-bit-truncated mantissa, R2 exact
    fp32 residual): fp32-grade precision at ~2x bf16 cost.
  - Spikes are exact 0/1 in f32r, so only weight rounding matters.
  - Elementwise split: DVE does STT/TT chain (t1, M, s, na), ACT does the
    -df*m^ scale-copy, POOL (gpsimd) does the m^ = M - s subtract.
  - Output pooling is folded into the PE: out_psum += W_out_tile @ s_t
    every step (single-pass f32r; ~1e-4 relative, well under tolerance).
  - Input matmul base = fused @ W_in runs once in honest fp32.
"""
import numpy as np

import concourse.bacc as bacc
import concourse.mybir as mybir
from concourse.tile import TileContext
from concourse.bass_utils import run_bass_kernel_spmd

B, F, H, O, T = 2048, 768, 1024, 16, 32
NCORES = 8
BC = B // NCORES          # 256 batch rows per core
KH = H // 128             # 8 h-tiles
KF = F // 128             # 6 f-tiles

f32 = mybir.dt.float32
f32r = mybir.dt.float32r

# fp32 arena columns
_WIN_W = KF * 128 * KH        # 6144
_FUSED_W = KF * BC            # 1536
_WOUT_W = KH * O              # 128 (fp32 W_out tiles for the epilogue matmul)
CF = _WIN_W + _FUSED_W + _WOUT_W
# f32r arena columns
_R_W = KH * KH * 128          # 8192
CR = 2 * _R_W                 # 16384
# vec columns: 9 sets of 8 + b_out col
CV = 80


LAST_EXEC_NS = None


def _trunc10(a):
    return (a.view(np.int32) & np.int32(~0x1FFF)).view(np.float32)


def _cols(v):
    """[1024] per-h vector -> [128, 8] (h-tile i in column i)."""
    return np.ascontiguousarray(v.reshape(KH, 128).T)


def _build_program(uniform_df, df_scalar):
    nc = bacc.Bacc(None, target_bir_lowering=False)
    d_af = nc.dram_tensor("af", [128, CF], f32, kind="ExternalInput")
    d_ar = nc.dram_tensor("ar", [128, CR], f32, kind="ExternalInput")
    d_vec = nc.dram_tensor("vec", [128, CV], f32, kind="ExternalInput")
    d_out = nc.dram_tensor("out", [O, BC], f32, kind="ExternalOutput")

    AL = mybir.AluOpType

    with TileContext(nc) as tc:
        with (
            tc.tile_pool(name="cpool", bufs=1) as cpool,
            tc.tile_pool(name="spool", bufs=2) as spool,
            tc.tile_pool(name="scr", bufs=8) as scr,
            tc.tile_pool(name="pb", bufs=5, space="PSUM") as pbp,
            tc.tile_pool(name="po", bufs=1, space="PSUM") as pop,
        ):
            af_t = cpool.tile([128, CF], f32, tag="af")
            ar_t = cpool.tile([128, CR], f32r, tag="ar")
            vec_t = cpool.tile([128, CV], f32, tag="vec")
            nc.sync.dma_start(vec_t[:], d_vec[:])
            nc.sync.dma_start(af_t[:], d_af[:])
            nc.sync.dma_start(ar_t[:], d_ar[:].bitcast(f32r))

            def vc(base, i):
                return vec_t[:, base + i:base + i + 1]

            def win(kf, i):
                c = (i * KF + kf) * 128
                return af_t[:, c:c + 128]

            def fusedT(kf):
                c = _WIN_W + kf * BC
                return af_t[:, c:c + BC]

            def rsplit(which, k, i):
                c = which * _R_W + (k * KH + i) * 128
                return ar_t[:, c:c + 128]

            def wof(k):
                c = _WIN_W + _FUSED_W + k * O
                return af_t[:, c:c + O]

            mh = [None] * KH
            na_l = [None] * KH
            s_l = [None] * KH
            th = [None] * KH

            po_t = pop.tile([O, BC], f32, tag="po")
            ssum = [None] * KH
            for i in range(KH):
                ssum[i] = cpool.tile([128, BC], f32, tag=f"ss{i}", name=f"ss{i}")
                nc.gpsimd.memset(ssum[i][:], 0.0)

            # ---- prologue: base = fused @ W_in (fp32), derive Theta/m^0/na0
            for i in range(KH):
                ps = pbp.tile([128, BC], f32, tag="ps")
                for kf in range(KF):
                    nc.tensor.matmul(ps[:], win(kf, i), fusedT(kf),
                                     start=(kf == 0), stop=(kf == KF - 1))
                th[i] = cpool.tile([128, BC], f32, tag=f"th{i}", name=f"th{i}")
                nc.vector.tensor_scalar(th[i][:], ps[:], vc(0, i), vc(8, i),
                                        AL.mult, AL.add)
                mh[i] = spool.tile([128, BC], f32, tag=f"mh{i}", name=f"mh{i}")
                nc.vector.tensor_scalar(mh[i][:], ps[:], vc(16, i), vc(24, i),
                                        AL.mult, AL.add)
                na_l[i] = spool.tile([128, BC], f32, tag=f"na{i}", name=f"na{i}")
                nc.vector.tensor_scalar(na_l[i][:], ps[:], vc(32, i), vc(40, i),
                                        AL.mult, AL.add)

            # ---- 32 unrolled steps
            for t in range(T):
                new_s = [None] * KH
                new_mh = [None] * KH
                new_na = [None] * KH
                for i in range(KH):
                    ps = None
                    if t > 0:
                        ps = pbp.tile([128, BC], f32, tag="ps")
                        mm = 0
                        for k in range(KH):
                            for which in (0, 1):
                                nc.tensor.matmul(ps[:], rsplit(which, k, i),
                                                 s_l[k][:],
                                                 start=(mm == 0),
                                                 stop=(mm == 2 * KH - 1))
                                mm += 1
                    t1 = scr.tile([128, BC], f32, tag="t1")
                    nc.vector.scalar_tensor_tensor(
                        t1[:], mh[i][:], vc(48, i), na_l[i][:],
                        AL.mult, AL.add)
                    if t > 0:
                        Mt = scr.tile([128, BC], f32, tag="M")
                        nc.vector.tensor_tensor(Mt[:], ps[:], t1[:], AL.add)
                    else:
                        Mt = t1
                    s_new = spool.tile([128, BC], f32r, tag=f"s{i}", name=f"s{i}_{t}")
                    nc.vector.tensor_tensor(s_new[:], Mt[:], th[i][:], AL.is_gt)
                    mh_new = spool.tile([128, BC], f32, tag=f"mh{i}", name=f"mh{i}_{t}")
                    nc.gpsimd.tensor_tensor(mh_new[:], Mt[:],
                                            s_new[:].bitcast(f32), AL.subtract)
                    t2 = scr.tile([128, BC], f32, tag="t2")
                    if uniform_df:
                        nc.scalar.activation(t2[:], mh[i][:],
                                             mybir.ActivationFunctionType.Copy,
                                             bias=0.0, scale=float(df_scalar))
                    else:
                        nc.scalar.activation(t2[:], mh[i][:],
                                             mybir.ActivationFunctionType.Copy,
                                             bias=0.0, scale=vc(64, i))
                    na_new = spool.tile([128, BC], f32, tag=f"na{i}", name=f"na{i}_{t}")
                    nc.vector.scalar_tensor_tensor(
                        na_new[:], na_l[i][:], vc(56, i), t2[:],
                        AL.mult, AL.add)
                    # accumulate spike counts on GpSimd (exact integers)
                    nc.gpsimd.tensor_tensor(ssum[i][:], ssum[i][:],
                                            s_new[:].bitcast(f32), AL.add)
                    new_s[i] = s_new
                    new_mh[i] = mh_new
                    new_na[i] = na_new
                s_l, mh, na_l = new_s, new_mh, new_na

            # ---- epilogue: out = (ssum @ W_out)/T + b_out (honest fp32)
            for k in range(KH):
                nc.tensor.matmul(po_t[:], wof(k), ssum[k][:],
                                 start=(k == 0), stop=(k == KH - 1))
            outt = cpool.tile([O, BC], f32, tag="outt")
            nc.vector.tensor_scalar(outt[:], po_t[:], 1.0 / T,
                                    vec_t[0:O, 72:73], AL.mult, AL.add)
            nc.sync.dma_start(d_out[:], outt[:])

    nc.compile()
    return nc


def kernel(fused, W_in, b_in, W_rec, b_rec, W_out, b_out,
           resonant_decay, resonant_frequency):
    fused = np.ascontiguousarray(fused, np.float32)
    W_in = np.ascontiguousarray(W_in, np.float32)
    W_rec = np.ascontiguousarray(W_rec, np.float32)
    W_out = np.ascontiguousarray(W_out, np.float32)
    b_in = np.asarray(b_in, np.float32)
    b_rec = np.asarray(b_rec, np.float32)
    b_out = np.asarray(b_out, np.float32)
    rd = np.asarray(resonant_decay, np.float32)
    rf = np.asarray(resonant_frequency, np.float32)

    one = np.float32(1.0)
    d = (np.float32(0.55) + np.float32(0.4) / (one + np.exp(-rd))).astype(np.float32)
    f = (np.float32(0.1) + np.float32(0.9) / (one + np.exp(-rf))).astype(np.float32)
    d64, f64 = d.astype(np.float64), f.astype(np.float64)
    den = (1.0 - d64) ** 2 + f64
    c1 = ((1.0 - d64) / den).astype(np.float32)
    c2 = (d64 * f64 / den).astype(np.float32)
    bb = (b_in + b_rec).astype(np.float32)

    vec_np = np.zeros((128, CV), np.float32)
    vec_np[:, 0:8] = _cols(-c1)
    vec_np[:, 8:16] = _cols(one - c1 * bb)
    vec_np[:, 16:24] = _cols(-c1)
    vec_np[:, 24:32] = _cols(-(c1 * bb))
    vec_np[:, 32:40] = _cols(c2)
    vec_np[:, 40:48] = _cols(c2 * bb)
    vec_np[:, 48:56] = _cols(d - f)
    vec_np[:, 56:64] = _cols(d)
    vec_np[:, 64:72] = _cols(-(d64 * f64).astype(np.float32))
    vec_np[0:O, 72] = b_out

    uniform_df = (np.ptp(d) == 0.0) and (np.ptp(f) == 0.0)
    df_scalar = float(-(d64[0] * f64[0])) if uniform_df else 0.0

    # f32r arena (same for all cores): W_rec residual split + W_out tiles
    R1 = _trunc10(W_rec)
    R2 = (W_rec - R1).astype(np.float32)
    ar_np = np.empty((128, CR), np.float32)
    for k in range(KH):
        for i in range(KH):
            ar_np[:, (k * KH + i) * 128:(k * KH + i + 1) * 128] = \
                R1[k * 128:(k + 1) * 128, i * 128:(i + 1) * 128]
            ar_np[:, _R_W + (k * KH + i) * 128:_R_W + (k * KH + i + 1) * 128] = \
                R2[k * 128:(k + 1) * 128, i * 128:(i + 1) * 128]

    # fp32 arena: W_in tiles (shared) + per-core transposed fused shard
    af_shared = np.empty((128, _WIN_W), np.float32)
    for i in range(KH):
        for kf in range(KF):
            af_shared[:, (i * KF + kf) * 128:(i * KF + kf + 1) * 128] = \
                W_in[kf * 128:(kf + 1) * 128, i * 128:(i + 1) * 128]

    nc = _build_program(uniform_df, df_scalar)

    in_maps = []
    for c in range(NCORES):
        af_np = np.empty((128, CF), np.float32)
        af_np[:, :_WIN_W] = af_shared
        shard = fused[c * BC:(c + 1) * BC, :]          # [256, 768]
        shardT = np.ascontiguousarray(shard.T)         # [768, 256]
        for kf in range(KF):
            af_np[:, _WIN_W + kf * BC:_WIN_W + (kf + 1) * BC] = \
                shardT[kf * 128:(kf + 1) * 128, :]
        for k in range(KH):
            af_np[:, _WIN_W + _FUSED_W + k * O:_WIN_W + _FUSED_W + (k + 1) * O] = \
                W_out[k * 128:(k + 1) * 128, :]
        in_maps.append({"af": af_np, "ar": ar_np, "vec": vec_np})

    res = run_bass_kernel_spmd(nc, in_maps, core_ids=list(range(NCORES)))
    global LAST_EXEC_NS
    LAST_EXEC_NS = res.exec_time_ns

    out_full = np.empty((B, O), np.float32)
    for c in range(NCORES):
        out_full[c * BC:(c + 1) * BC, :] = res.results[c]["out"].T
    return out_full


# revision 8
# speedup vs baseline: 1.1523x; 1.1523x over previous
"""Trainium2 Bass kernel for nn_ExperimentalPathwayModel (resonant spiking RNN).

Strategy (data-parallel over batch, 8 cores x 256 rows, H-major layout
[128 h-partitions x 256 batch free] per tile):

  - The affine state shift m = m^ + p, r = r^ + q (p = base(1-d)/((1-d)^2+f),
    q = f p/(1-d)) eliminates the per-step "+ base" drive from the loop.
    With na = -d*r^ the per-step math is
        M    = (d-f) m^ + na + rec        (rec = s_{t-1} @ W_rec)
        s    = 1[M > Theta],  Theta = 1 - p
        m^'  = M - s
        na'  = d na - (d f) m^
  - W_rec matmul runs on the PE in float32r (1 cyc/row) as a 2-pass
    residual split W = R1 + R2 (R1 = 10-bit-truncated mantissa, R2 exact
    fp32 residual): fp32-grade precision at ~2x bf16 cost.
  - Spikes are exact 0/1 in f32r, so only weight rounding matters.
  - Elementwise split: DVE does STT/TT chain (t1, M, s, na), ACT does the
    -df*m^ scale-copy, POOL (gpsimd) does the m^ = M - s subtract.
  - Output pooling is folded into the PE: out_psum += W_out_tile @ s_t
    every step (single-pass f32r; ~1e-4 relative, well under tolerance).
  - Input matmul base = fused @ W_in runs once in honest fp32.
"""
import numpy as np

import concourse.bacc as bacc
import concourse.mybir as mybir
from concourse.tile import TileContext
from concourse.bass_utils import run_bass_kernel_spmd

B, F, H, O, T = 2048, 768, 1024, 16, 32
NCORES = 8
BC = B // NCORES          # 256 batch rows per core
KH = H // 128             # 8 h-tiles
KF = F // 128             # 6 f-tiles

f32 = mybir.dt.float32
f32r = mybir.dt.float32r

# fp32 arena columns
_WIN_W = KF * 128 * KH        # 6144
_FUSED_W = KF * BC            # 1536
_WOUT_W = KH * O              # 128 (fp32 W_out tiles for the epilogue matmul)
CF = _WIN_W + _FUSED_W + _WOUT_W
# f32r arena columns
_R_W = KH * KH * 128          # 8192
CR = 2 * _R_W                 # 16384
# vec columns: 9 sets of 8 + b_out col
CV = 80


LAST_EXEC_NS = None


def _trunc10(a):
    return (a.view(np.int32) & np.int32(~0x1FFF)).view(np.float32)


def _cols(v):
    """[1024] per-h vector -> [128, 8] (h-tile i in column i)."""
    return np.ascontiguousarray(v.reshape(KH, 128).T)


def _build_program(uniform_df, df_vals):
    nc = bacc.Bacc(None, target_bir_lowering=False)
    d_af = nc.dram_tensor("af", [128, CF], f32, kind="ExternalInput")
    d_ar = nc.dram_tensor("ar", [128, CR], f32, kind="ExternalInput")
    d_vec = nc.dram_tensor("vec", [128, CV], f32, kind="ExternalInput")
    d_out = nc.dram_tensor("out", [O, BC], f32, kind="ExternalOutput")

    AL = mybir.AluOpType

    with TileContext(nc) as tc:
        with (
            tc.tile_pool(name="cpool", bufs=1) as cpool,
            tc.tile_pool(name="spool", bufs=2) as spool,
            tc.tile_pool(name="scr", bufs=8) as scr,
            tc.tile_pool(name="pb", bufs=5, space="PSUM") as pbp,
            tc.tile_pool(name="po", bufs=1, space="PSUM") as pop,
        ):
            af_t = cpool.tile([128, CF], f32, tag="af")
            ar_t = cpool.tile([128, CR], f32r, tag="ar")
            vec_t = cpool.tile([128, CV], f32, tag="vec")
            nc.sync.dma_start(vec_t[:], d_vec[:])
            nc.sync.dma_start(af_t[:], d_af[:])
            nc.sync.dma_start(ar_t[:], d_ar[:].bitcast(f32r))

            def vc(base, i):
                return vec_t[:, base + i:base + i + 1]

            def win(kf, i):
                c = (i * KF + kf) * 128
                return af_t[:, c:c + 128]

            def fusedT(kf):
                c = _WIN_W + kf * BC
                return af_t[:, c:c + BC]

            def rsplit(which, k, i):
                c = which * _R_W + (k * KH + i) * 128
                return ar_t[:, c:c + 128]

            def wof(k):
                c = _WIN_W + _FUSED_W + k * O
                return af_t[:, c:c + O]

            mh = [None] * KH
            na_l = [None] * KH
            s_l = [None] * KH
            th = [None] * KH

            po_t = pop.tile([O, BC], f32, tag="po")
            ssum = [None] * KH
            for i in range(KH):
                ssum[i] = cpool.tile([128, BC], f32, tag=f"ss{i}", name=f"ss{i}")
                nc.gpsimd.memset(ssum[i][:], 0.0)

            HB = KH * BC  # 2048-wide state tiles (uniform path)
            if uniform_df:
                th_b = cpool.tile([128, HB], f32, tag="th_b")
                mh_b = spool.tile([128, HB], f32, tag="mh_b", name="mh_b0")
                na_b = spool.tile([128, HB], f32, tag="na_b", name="na_b0")

            # ---- prologue: base = fused @ W_in (fp32), derive Theta/m^0/na0
            for i in range(KH):
                ps = pbp.tile([128, BC], f32, tag="ps")
                for kf in range(KF):
                    nc.tensor.matmul(ps[:], win(kf, i), fusedT(kf),
                                     start=(kf == 0), stop=(kf == KF - 1))
                if uniform_df:
                    sl = slice(i * BC, (i + 1) * BC)
                    nc.vector.tensor_scalar(th_b[:, sl], ps[:], vc(0, i), vc(8, i),
                                            AL.mult, AL.add)
                    nc.vector.tensor_scalar(mh_b[:, sl], ps[:], vc(16, i), vc(24, i),
                                            AL.mult, AL.add)
                    nc.vector.tensor_scalar(na_b[:, sl], ps[:], vc(32, i), vc(40, i),
                                            AL.mult, AL.add)
                else:
                    th[i] = cpool.tile([128, BC], f32, tag=f"th{i}", name=f"th{i}")
                    nc.vector.tensor_scalar(th[i][:], ps[:], vc(0, i), vc(8, i),
                                            AL.mult, AL.add)
                    mh[i] = spool.tile([128, BC], f32, tag=f"mh{i}", name=f"mh{i}")
                    nc.vector.tensor_scalar(mh[i][:], ps[:], vc(16, i), vc(24, i),
                                            AL.mult, AL.add)
                    na_l[i] = spool.tile([128, BC], f32, tag=f"na{i}", name=f"na{i}")
                    nc.vector.tensor_scalar(na_l[i][:], ps[:], vc(32, i), vc(40, i),
                                            AL.mult, AL.add)

            if uniform_df:
                # ==== wide-op path: scalars are immediates, all 8 h-tiles in
                # one instruction for off-psum ops. s = 1[psum > U], U = Th - t1.
                c_dmf = float(df_vals[0])   # d - f
                c_d = float(df_vals[1])     # d
                c_ndf = float(df_vals[2])   # -d*f
                for t in range(T):
                    t1_b = scr.tile([128, HB], f32, tag="t1b", bufs=1)
                    nc.vector.scalar_tensor_tensor(
                        t1_b[:], mh_b[:], c_dmf, na_b[:], AL.mult, AL.add)
                    u_b = scr.tile([128, HB], f32, tag="ub", bufs=1)
                    nc.vector.tensor_tensor(u_b[:], th_b[:], t1_b[:], AL.subtract)
                    t2_b = scr.tile([128, HB], f32, tag="t2b", bufs=1)
                    nc.scalar.activation(t2_b[:], mh_b[:],
                                         mybir.ActivationFunctionType.Copy,
                                         bias=0.0, scale=c_ndf)
                    na_new = spool.tile([128, HB], f32, tag="na_b", name=f"na_b{t+1}")
                    nc.vector.scalar_tensor_tensor(
                        na_new[:], na_b[:], c_d, t2_b[:], AL.mult, AL.add)
                    mh_new = spool.tile([128, HB], f32, tag="mh_b", name=f"mh_b{t+1}")
                    new_s = [None] * KH
                    for i in range(KH):
                        sl = slice(i * BC, (i + 1) * BC)
                        ps = None
                        if t > 0:
                            ps = pbp.tile([128, BC], f32, tag="ps")
                            mm = 0
                            for k in range(KH):
                                for which in (0, 1):
                                    nc.tensor.matmul(ps[:], rsplit(which, k, i),
                                                     s_l[k][:],
                                                     start=(mm == 0),
                                                     stop=(mm == 2 * KH - 1))
                                    mm += 1
                        s_new = spool.tile([128, BC], f32r, tag=f"s{i}",
                                           name=f"s{i}_{t}")
                        if t > 0:
                            nc.vector.tensor_tensor(s_new[:], ps[:], u_b[:, sl],
                                                    AL.is_gt)
                            Mt = scr.tile([128, BC], f32, tag="M", bufs=4)
                            nc.vector.tensor_tensor(Mt[:], ps[:], t1_b[:, sl],
                                                    AL.add)
                        else:
                            # no rec at t=0: psum == 0, so s = 1[0 > U], M = t1
                            nc.vector.tensor_scalar(s_new[:], u_b[:, sl], 0.0, None,
                                                    AL.is_lt)
                            Mt = None
                        src_m = Mt[:] if Mt is not None else t1_b[:, sl]
                        nc.gpsimd.tensor_tensor(mh_new[:, sl], src_m,
                                                s_new[:].bitcast(f32), AL.subtract)
                        nc.gpsimd.tensor_tensor(ssum[i][:], ssum[i][:],
                                                s_new[:].bitcast(f32), AL.add)
                        new_s[i] = s_new
                    s_l = new_s
                    mh_b = mh_new
                    na_b = na_new
            else:
                # ==== general per-tile path (per-partition scalar APs)
                for t in range(T):
                    new_s = [None] * KH
                    new_mh = [None] * KH
                    new_na = [None] * KH
                    for i in range(KH):
                        ps = None
                        if t > 0:
                            ps = pbp.tile([128, BC], f32, tag="ps")
                            mm = 0
                            for k in range(KH):
                                for which in (0, 1):
                                    nc.tensor.matmul(ps[:], rsplit(which, k, i),
                                                     s_l[k][:],
                                                     start=(mm == 0),
                                                     stop=(mm == 2 * KH - 1))
                                    mm += 1
                        t1 = scr.tile([128, BC], f32, tag="t1")
                        nc.vector.scalar_tensor_tensor(
                            t1[:], mh[i][:], vc(48, i), na_l[i][:],
                            AL.mult, AL.add)
                        u_t = scr.tile([128, BC], f32, tag="u")
                        nc.vector.tensor_tensor(u_t[:], th[i][:], t1[:],
                                                AL.subtract)
                        s_new = spool.tile([128, BC], f32r, tag=f"s{i}",
                                           name=f"s{i}_{t}")
                        if t > 0:
                            nc.vector.tensor_tensor(s_new[:], ps[:], u_t[:],
                                                    AL.is_gt)
                            Mt = scr.tile([128, BC], f32, tag="M")
                            nc.vector.tensor_tensor(Mt[:], ps[:], t1[:], AL.add)
                        else:
                            nc.vector.tensor_scalar(s_new[:], u_t[:], 0.0, None,
                                                    AL.is_lt)
                            Mt = t1
                        mh_new = spool.tile([128, BC], f32, tag=f"mh{i}",
                                            name=f"mh{i}_{t}")
                        nc.gpsimd.tensor_tensor(mh_new[:], Mt[:],
                                                s_new[:].bitcast(f32), AL.subtract)
                        t2 = scr.tile([128, BC], f32, tag="t2")
                        nc.scalar.activation(t2[:], mh[i][:],
                                             mybir.ActivationFunctionType.Copy,
                                             bias=0.0, scale=vc(64, i))
                        na_new = spool.tile([128, BC], f32, tag=f"na{i}",
                                            name=f"na{i}_{t}")
                        nc.vector.scalar_tensor_tensor(
                            na_new[:], na_l[i][:], vc(56, i), t2[:],
                            AL.mult, AL.add)
                        nc.gpsimd.tensor_tensor(ssum[i][:], ssum[i][:],
                                                s_new[:].bitcast(f32), AL.add)
                        new_s[i] = s_new
                        new_mh[i] = mh_new
                        new_na[i] = na_new
                    s_l, mh, na_l = new_s, new_mh, new_na

            # ---- epilogue: out = (ssum @ W_out)/T + b_out (honest fp32)
            for k in range(KH):
                nc.tensor.matmul(po_t[:], wof(k), ssum[k][:],
                                 start=(k == 0), stop=(k == KH - 1))
            outt = cpool.tile([O, BC], f32, tag="outt")
            nc.vector.tensor_scalar(outt[:], po_t[:], 1.0 / T,
                                    vec_t[0:O, 72:73], AL.mult, AL.add)
            nc.sync.dma_start(d_out[:], outt[:])

    nc.compile()
    return nc


def kernel(fused, W_in, b_in, W_rec, b_rec, W_out, b_out,
           resonant_decay, resonant_frequency):
    fused = np.ascontiguousarray(fused, np.float32)
    W_in = np.ascontiguousarray(W_in, np.float32)
    W_rec = np.ascontiguousarray(W_rec, np.float32)
    W_out = np.ascontiguousarray(W_out, np.float32)
    b_in = np.asarray(b_in, np.float32)
    b_rec = np.asarray(b_rec, np.float32)
    b_out = np.asarray(b_out, np.float32)
    rd = np.asarray(resonant_decay, np.float32)
    rf = np.asarray(resonant_frequency, np.float32)

    one = np.float32(1.0)
    d = (np.float32(0.55) + np.float32(0.4) / (one + np.exp(-rd))).astype(np.float32)
    f = (np.float32(0.1) + np.float32(0.9) / (one + np.exp(-rf))).astype(np.float32)
    d64, f64 = d.astype(np.float64), f.astype(np.float64)
    den = (1.0 - d64) ** 2 + f64
    c1 = ((1.0 - d64) / den).astype(np.float32)
    c2 = (d64 * f64 / den).astype(np.float32)
    bb = (b_in + b_rec).astype(np.float32)

    vec_np = np.zeros((128, CV), np.float32)
    vec_np[:, 0:8] = _cols(-c1)
    vec_np[:, 8:16] = _cols(one - c1 * bb)
    vec_np[:, 16:24] = _cols(-c1)
    vec_np[:, 24:32] = _cols(-(c1 * bb))
    vec_np[:, 32:40] = _cols(c2)
    vec_np[:, 40:48] = _cols(c2 * bb)
    vec_np[:, 48:56] = _cols(d - f)
    vec_np[:, 56:64] = _cols(d)
    vec_np[:, 64:72] = _cols(-(d64 * f64).astype(np.float32))
    vec_np[0:O, 72] = b_out

    uniform_df = (np.ptp(d) == 0.0) and (np.ptp(f) == 0.0)
    df_vals = (float(d[0] - f[0]), float(d[0]),
               float(-(d64[0] * f64[0]))) if uniform_df else (0.0, 0.0, 0.0)

    # f32r arena (same for all cores): W_rec residual split + W_out tiles
    R1 = _trunc10(W_rec)
    R2 = (W_rec - R1).astype(np.float32)
    ar_np = np.empty((128, CR), np.float32)
    for k in range(KH):
        for i in range(KH):
            ar_np[:, (k * KH + i) * 128:(k * KH + i + 1) * 128] = \
                R1[k * 128:(k + 1) * 128, i * 128:(i + 1) * 128]
            ar_np[:, _R_W + (k * KH + i) * 128:_R_W + (k * KH + i + 1) * 128] = \
                R2[k * 128:(k + 1) * 128, i * 128:(i + 1) * 128]

    # fp32 arena: W_in tiles (shared) + per-core transposed fused shard
    af_shared = np.empty((128, _WIN_W), np.float32)
    for i in range(KH):
        for kf in range(KF):
            af_shared[:, (i * KF + kf) * 128:(i * KF + kf + 1) * 128] = \
                W_in[kf * 128:(kf + 1) * 128, i * 128:(i + 1) * 128]

    nc = _build_program(uniform_df, df_vals)

    in_maps = []
    for c in range(NCORES):
        af_np = np.empty((128, CF), np.float32)
        af_np[:, :_WIN_W] = af_shared
        shard = fused[c * BC:(c + 1) * BC, :]          # [256, 768]
        shardT = np.ascontiguousarray(shard.T)         # [768, 256]
        for kf in range(KF):
            af_np[:, _WIN_W + kf * BC:_WIN_W + (kf + 1) * BC] = \
                shardT[kf * 128:(kf + 1) * 128, :]
        for k in range(KH):
            af_np[:, _WIN_W + _FUSED_W + k * O:_WIN_W + _FUSED_W + (k + 1) * O] = \
                W_out[k * 128:(k + 1) * 128, :]
        in_maps.append({"af": af_np, "ar": ar_np, "vec": vec_np})

    res = run_bass_kernel_spmd(nc, in_maps, core_ids=list(range(NCORES)))
    global LAST_EXEC_NS
    LAST_EXEC_NS = res.exec_time_ns

    out_full = np.empty((B, O), np.float32)
    for c in range(NCORES):
        out_full[c * BC:(c + 1) * BC, :] = res.results[c]["out"].T
    return out_full


# revision 9
# speedup vs baseline: 1.1980x; 1.0397x over previous
"""Trainium2 Bass kernel for nn_ExperimentalPathwayModel (resonant spiking RNN).

Strategy (data-parallel over batch, 8 cores x 256 rows, H-major layout
[128 h-partitions x 256 batch free] per tile):

  - The affine state shift m = m^ + p, r = r^ + q (p = base(1-d)/((1-d)^2+f),
    q = f p/(1-d)) eliminates the per-step "+ base" drive from the loop.
    With na = -d*r^ the per-step math is
        M    = (d-f) m^ + na + rec        (rec = s_{t-1} @ W_rec)
        s    = 1[M > Theta],  Theta = 1 - p
        m^'  = M - s
        na'  = d na - (d f) m^
  - W_rec matmul runs on the PE in float32r (1 cyc/row) as a 2-pass
    residual split W = R1 + R2 (R1 = 10-bit-truncated mantissa, R2 exact
    fp32 residual): fp32-grade precision at ~2x bf16 cost.
  - Spikes are exact 0/1 in f32r, so only weight rounding matters.
  - Elementwise split: DVE does STT/TT chain (t1, M, s, na), ACT does the
    -df*m^ scale-copy, POOL (gpsimd) does the m^ = M - s subtract.
  - Output pooling is folded into the PE: out_psum += W_out_tile @ s_t
    every step (single-pass f32r; ~1e-4 relative, well under tolerance).
  - Input matmul base = fused @ W_in runs once in honest fp32.
"""
import numpy as np

import concourse.bacc as bacc
import concourse.mybir as mybir
from concourse.tile import TileContext
from concourse.bass_utils import run_bass_kernel_spmd

B, F, H, O, T = 2048, 768, 1024, 16, 32
NCORES = 8
BC = B // NCORES          # 256 batch rows per core
KH = H // 128             # 8 h-tiles
KF = F // 128             # 6 f-tiles

f32 = mybir.dt.float32
f32r = mybir.dt.float32r

# fp32 arena columns
_WIN_W = KF * 128 * KH        # 6144
_FUSED_W = KF * BC            # 1536
_WOUT_W = KH * O              # 128 (fp32 W_out tiles for the epilogue matmul)
CF = _WIN_W + _FUSED_W + _WOUT_W
# f32r arena columns
_R_W = KH * KH * 128          # 8192
CR = 2 * _R_W                 # 16384
# vec columns: 9 sets of 8 + b_out col
CV = 80


LAST_EXEC_NS = None


def _trunc10(a):
    return (a.view(np.int32) & np.int32(~0x1FFF)).view(np.float32)


def _cols(v):
    """[1024] per-h vector -> [128, 8] (h-tile i in column i)."""
    return np.ascontiguousarray(v.reshape(KH, 128).T)


def _build_program(uniform_df, df_vals):
    nc = bacc.Bacc(None, target_bir_lowering=False)
    d_af = nc.dram_tensor("af", [128, CF], f32, kind="ExternalInput")
    d_ar = nc.dram_tensor("ar", [128, CR], f32, kind="ExternalInput")
    d_vec = nc.dram_tensor("vec", [128, CV], f32, kind="ExternalInput")
    d_out = nc.dram_tensor("out", [O, BC], f32, kind="ExternalOutput")

    AL = mybir.AluOpType

    with TileContext(nc) as tc:
        with (
            tc.tile_pool(name="cpool", bufs=1) as cpool,
            tc.tile_pool(name="spool", bufs=2) as spool,
            tc.tile_pool(name="scr", bufs=8) as scr,
            tc.tile_pool(name="pb", bufs=6, space="PSUM") as pbp,
            tc.tile_pool(name="po", bufs=1, space="PSUM") as pop,
        ):
            af_t = cpool.tile([128, CF], f32, tag="af")
            ar_t = cpool.tile([128, CR], f32r, tag="ar")
            vec_t = cpool.tile([128, CV], f32, tag="vec")
            nc.sync.dma_start(vec_t[:], d_vec[:])
            nc.sync.dma_start(af_t[:], d_af[:])
            nc.sync.dma_start(ar_t[:], d_ar[:].bitcast(f32r))

            def vc(base, i):
                return vec_t[:, base + i:base + i + 1]

            def win(kf, i):
                c = (i * KF + kf) * 128
                return af_t[:, c:c + 128]

            def fusedT(kf):
                c = _WIN_W + kf * BC
                return af_t[:, c:c + BC]

            def rsplit(which, k, i):
                c = which * _R_W + (k * KH + i) * 128
                return ar_t[:, c:c + 128]

            def wof(k):
                c = _WIN_W + _FUSED_W + k * O
                return af_t[:, c:c + O]

            mh = [None] * KH
            na_l = [None] * KH
            s_l = [None] * KH
            th = [None] * KH

            po_t = pop.tile([O, BC], f32, tag="po")
            ssum = [None] * KH
            for i in range(KH):
                ssum[i] = cpool.tile([128, BC], f32, tag=f"ss{i}", name=f"ss{i}")
                nc.gpsimd.memset(ssum[i][:], 0.0)

            HB = KH * BC  # 2048-wide state tiles (uniform path)
            if uniform_df:
                th_b = cpool.tile([128, HB], f32, tag="th_b")
                mh_b = spool.tile([128, HB], f32, tag="mh_b", name="mh_b0")
                na_b = spool.tile([128, HB], f32, tag="na_b", name="na_b0")

            # ---- prologue: base = fused @ W_in (fp32), derive Theta/m^0/na0
            for i in range(KH):
                ps = pbp.tile([128, BC], f32, tag="ps")
                for kf in range(KF):
                    nc.tensor.matmul(ps[:], win(kf, i), fusedT(kf),
                                     start=(kf == 0), stop=(kf == KF - 1))
                if uniform_df:
                    sl = slice(i * BC, (i + 1) * BC)
                    nc.vector.tensor_scalar(th_b[:, sl], ps[:], vc(0, i), vc(8, i),
                                            AL.mult, AL.add)
                    nc.vector.tensor_scalar(mh_b[:, sl], ps[:], vc(16, i), vc(24, i),
                                            AL.mult, AL.add)
                    nc.vector.tensor_scalar(na_b[:, sl], ps[:], vc(32, i), vc(40, i),
                                            AL.mult, AL.add)
                else:
                    th[i] = cpool.tile([128, BC], f32, tag=f"th{i}", name=f"th{i}")
                    nc.vector.tensor_scalar(th[i][:], ps[:], vc(0, i), vc(8, i),
                                            AL.mult, AL.add)
                    mh[i] = spool.tile([128, BC], f32, tag=f"mh{i}", name=f"mh{i}")
                    nc.vector.tensor_scalar(mh[i][:], ps[:], vc(16, i), vc(24, i),
                                            AL.mult, AL.add)
                    na_l[i] = spool.tile([128, BC], f32, tag=f"na{i}", name=f"na{i}")
                    nc.vector.tensor_scalar(na_l[i][:], ps[:], vc(32, i), vc(40, i),
                                            AL.mult, AL.add)

            if uniform_df:
                # ==== wide-op path: scalars are immediates, all 8 h-tiles in
                # one instruction for off-psum ops. s = 1[psum > U], U = Th - t1.
                c_dmf = float(df_vals[0])   # d - f
                c_d = float(df_vals[1])     # d
                c_ndf = float(df_vals[2])   # -d*f
                QW = 2 * BC  # quarter width: 2 h-tiles per t1/u op
                for t in range(T):
                    # t1/u in quarters so each unlocks as its mh slices land
                    t1_b = scr.tile([128, HB], f32, tag="t1b", bufs=1)
                    u_b = scr.tile([128, HB], f32, tag="ub", bufs=1)
                    for qj in range(4):
                        qs = slice(qj * QW, (qj + 1) * QW)
                        nc.vector.scalar_tensor_tensor(
                            t1_b[:, qs], mh_b[:, qs], c_dmf, na_b[:, qs],
                            AL.mult, AL.add)
                        nc.vector.tensor_tensor(u_b[:, qs], th_b[:, qs],
                                                t1_b[:, qs], AL.subtract)
                    t2_b = scr.tile([128, HB], f32, tag="t2b", bufs=1)
                    nc.scalar.activation(t2_b[:], mh_b[:],
                                         mybir.ActivationFunctionType.Copy,
                                         bias=0.0, scale=c_ndf)
                    na_new = spool.tile([128, HB], f32, tag="na_b", name=f"na_b{t+1}")
                    nc.vector.scalar_tensor_tensor(
                        na_new[:], na_b[:], c_d, t2_b[:], AL.mult, AL.add)
                    mh_new = spool.tile([128, HB], f32, tag="mh_b", name=f"mh_b{t+1}")
                    new_s = [None] * KH
                    for i in range(KH):
                        sl = slice(i * BC, (i + 1) * BC)
                        ps = None
                        if t > 0:
                            ps = pbp.tile([128, BC], f32, tag="ps")
                            mm = 0
                            for k in range(KH):
                                for which in (0, 1):
                                    nc.tensor.matmul(ps[:], rsplit(which, k, i),
                                                     s_l[k][:],
                                                     start=(mm == 0),
                                                     stop=(mm == 2 * KH - 1))
                                    mm += 1
                        s_new = spool.tile([128, BC], f32r, tag=f"s{i}",
                                           name=f"s{i}_{t}")
                        if t > 0:
                            nc.vector.tensor_tensor(s_new[:], ps[:], u_b[:, sl],
                                                    AL.is_gt)
                            Mt = scr.tile([128, BC], f32, tag="M", bufs=4)
                            nc.vector.tensor_tensor(Mt[:], ps[:], t1_b[:, sl],
                                                    AL.add)
                        else:
                            # no rec at t=0: psum == 0, so s = 1[0 > U], M = t1
                            nc.vector.tensor_scalar(s_new[:], u_b[:, sl], 0.0, None,
                                                    AL.is_lt)
                            Mt = None
                        src_m = Mt[:] if Mt is not None else t1_b[:, sl]
                        nc.gpsimd.tensor_tensor(mh_new[:, sl], src_m,
                                                s_new[:].bitcast(f32), AL.subtract)
                        new_s[i] = s_new
                    # spike-count accumulation off the critical path
                    for i in range(KH):
                        nc.gpsimd.tensor_tensor(ssum[i][:], ssum[i][:],
                                                new_s[i][:].bitcast(f32), AL.add)
                    s_l = new_s
                    mh_b = mh_new
                    na_b = na_new
            else:
                # ==== general per-tile path (per-partition scalar APs)
                for t in range(T):
                    new_s = [None] * KH
                    new_mh = [None] * KH
                    new_na = [None] * KH
                    for i in range(KH):
                        ps = None
                        if t > 0:
                            ps = pbp.tile([128, BC], f32, tag="ps")
                            mm = 0
                            for k in range(KH):
                                for which in (0, 1):
                                    nc.tensor.matmul(ps[:], rsplit(which, k, i),
                                                     s_l[k][:],
                                                     start=(mm == 0),
                                                     stop=(mm == 2 * KH - 1))
                                    mm += 1
                        t1 = scr.tile([128, BC], f32, tag="t1")
                        nc.vector.scalar_tensor_tensor(
                            t1[:], mh[i][:], vc(48, i), na_l[i][:],
                            AL.mult, AL.add)
                        u_t = scr.tile([128, BC], f32, tag="u")
                        nc.vector.tensor_tensor(u_t[:], th[i][:], t1[:],
                                                AL.subtract)
                        s_new = spool.tile([128, BC], f32r, tag=f"s{i}",
                                           name=f"s{i}_{t}")
                        if t > 0:
                            nc.vector.tensor_tensor(s_new[:], ps[:], u_t[:],
                                                    AL.is_gt)
                            Mt = scr.tile([128, BC], f32, tag="M")
                            nc.vector.tensor_tensor(Mt[:], ps[:], t1[:], AL.add)
                        else:
                            nc.vector.tensor_scalar(s_new[:], u_t[:], 0.0, None,
                                                    AL.is_lt)
                            Mt = t1
                        mh_new = spool.tile([128, BC], f32, tag=f"mh{i}",
                                            name=f"mh{i}_{t}")
                        nc.gpsimd.tensor_tensor(mh_new[:], Mt[:],
                                                s_new[:].bitcast(f32), AL.subtract)
                        t2 = scr.tile([128, BC], f32, tag="t2")
                        nc.scalar.activation(t2[:], mh[i][:],
                                             mybir.ActivationFunctionType.Copy,
                                             bias=0.0, scale=vc(64, i))
                        na_new = spool.tile([128, BC], f32, tag=f"na{i}",
                                            name=f"na{i}_{t}")
                        nc.vector.scalar_tensor_tensor(
                            na_new[:], na_l[i][:], vc(56, i), t2[:],
                            AL.mult, AL.add)
                        nc.gpsimd.tensor_tensor(ssum[i][:], ssum[i][:],
                                                s_new[:].bitcast(f32), AL.add)
                        new_s[i] = s_new
                        new_mh[i] = mh_new
                        new_na[i] = na_new
                    s_l, mh, na_l = new_s, new_mh, new_na

            # ---- epilogue: out = (ssum @ W_out)/T + b_out (honest fp32)
            for k in range(KH):
                nc.tensor.matmul(po_t[:], wof(k), ssum[k][:],
                                 start=(k == 0), stop=(k == KH - 1))
            outt = cpool.tile([O, BC], f32, tag="outt")
            nc.vector.tensor_scalar(outt[:], po_t[:], 1.0 / T,
                                    vec_t[0:O, 72:73], AL.mult, AL.add)
            nc.sync.dma_start(d_out[:], outt[:])

    nc.compile()
    return nc


def kernel(fused, W_in, b_in, W_rec, b_rec, W_out, b_out,
           resonant_decay, resonant_frequency):
    fused = np.ascontiguousarray(fused, np.float32)
    W_in = np.ascontiguousarray(W_in, np.float32)
    W_rec = np.ascontiguousarray(W_rec, np.float32)
    W_out = np.ascontiguousarray(W_out, np.float32)
    b_in = np.asarray(b_in, np.float32)
    b_rec = np.asarray(b_rec, np.float32)
    b_out = np.asarray(b_out, np.float32)
    rd = np.asarray(resonant_decay, np.float32)
    rf = np.asarray(resonant_frequency, np.float32)

    one = np.float32(1.0)
    d = (np.float32(0.55) + np.float32(0.4) / (one + np.exp(-rd))).astype(np.float32)
    f = (np.float32(0.1) + np.float32(0.9) / (one + np.exp(-rf))).astype(np.float32)
    d64, f64 = d.astype(np.float64), f.astype(np.float64)
    den = (1.0 - d64) ** 2 + f64
    c1 = ((1.0 - d64) / den).astype(np.float32)
    c2 = (d64 * f64 / den).astype(np.float32)
    bb = (b_in + b_rec).astype(np.float32)

    vec_np = np.zeros((128, CV), np.float32)
    vec_np[:, 0:8] = _cols(-c1)
    vec_np[:, 8:16] = _cols(one - c1 * bb)
    vec_np[:, 16:24] = _cols(-c1)
    vec_np[:, 24:32] = _cols(-(c1 * bb))
    vec_np[:, 32:40] = _cols(c2)
    vec_np[:, 40:48] = _cols(c2 * bb)
    vec_np[:, 48:56] = _cols(d - f)
    vec_np[:, 56:64] = _cols(d)
    vec_np[:, 64:72] = _cols(-(d64 * f64).astype(np.float32))
    vec_np[0:O, 72] = b_out

    uniform_df = (np.ptp(d) == 0.0) and (np.ptp(f) == 0.0)
    df_vals = (float(d[0] - f[0]), float(d[0]),
               float(-(d64[0] * f64[0]))) if uniform_df else (0.0, 0.0, 0.0)

    # f32r arena (same for all cores): W_rec residual split + W_out tiles
    R1 = _trunc10(W_rec)
    R2 = (W_rec - R1).astype(np.float32)
    ar_np = np.empty((128, CR), np.float32)
    for k in range(KH):
        for i in range(KH):
            ar_np[:, (k * KH + i) * 128:(k * KH + i + 1) * 128] = \
                R1[k * 128:(k + 1) * 128, i * 128:(i + 1) * 128]
            ar_np[:, _R_W + (k * KH + i) * 128:_R_W + (k * KH + i + 1) * 128] = \
                R2[k * 128:(k + 1) * 128, i * 128:(i + 1) * 128]

    # fp32 arena: W_in tiles (shared) + per-core transposed fused shard
    af_shared = np.empty((128, _WIN_W), np.float32)
    for i in range(KH):
        for kf in range(KF):
            af_shared[:, (i * KF + kf) * 128:(i * KF + kf + 1) * 128] = \
                W_in[kf * 128:(kf + 1) * 128, i * 128:(i + 1) * 128]

    nc = _build_program(uniform_df, df_vals)

    in_maps = []
    for c in range(NCORES):
        af_np = np.empty((128, CF), np.float32)
        af_np[:, :_WIN_W] = af_shared
        shard = fused[c * BC:(c + 1) * BC, :]          # [256, 768]
        shardT = np.ascontiguousarray(shard.T)         # [768, 256]
        for kf in range(KF):
            af_np[:, _WIN_W + kf * BC:_WIN_W + (kf + 1) * BC] = \
                shardT[kf * 128:(kf + 1) * 128, :]
        for k in range(KH):
            af_np[:, _WIN_W + _FUSED_W + k * O:_WIN_W + _FUSED_W + (k + 1) * O] = \
                W_out[k * 128:(k + 1) * 128, :]
        in_maps.append({"af": af_np, "ar": ar_np, "vec": vec_np})

    res = run_bass_kernel_spmd(nc, in_maps, core_ids=list(range(NCORES)))
    global LAST_EXEC_NS
    LAST_EXEC_NS = res.exec_time_ns

    out_full = np.empty((B, O), np.float32)
    for c in range(NCORES):
        out_full[c * BC:(c + 1) * BC, :] = res.results[c]["out"].T
    return out_full


# revision 10
# speedup vs baseline: 1.2205x; 1.0188x over previous
"""Trainium2 Bass kernel for nn_ExperimentalPathwayModel (resonant spiking RNN).

Strategy (data-parallel over batch, 8 cores x 256 rows, H-major layout
[128 h-partitions x 256 batch free] per tile):

  - The affine state shift m = m^ + p, r = r^ + q (p = base(1-d)/((1-d)^2+f),
    q = f p/(1-d)) eliminates the per-step "+ base" drive from the loop.
    With na = -d*r^ the per-step math is
        M    = (d-f) m^ + na + rec        (rec = s_{t-1} @ W_rec)
        s    = 1[M > Theta],  Theta = 1 - p
        m^'  = M - s
        na'  = d na - (d f) m^
  - W_rec matmul runs on the PE in float32r (1 cyc/row) as a 2-pass
    residual split W = R1 + R2 (R1 = 10-bit-truncated mantissa, R2 exact
    fp32 residual): fp32-grade precision at ~2x bf16 cost.
  - Spikes are exact 0/1 in f32r, so only weight rounding matters.
  - Elementwise split: DVE does STT/TT chain (t1, M, s, na), ACT does the
    -df*m^ scale-copy, POOL (gpsimd) does the m^ = M - s subtract.
  - Output pooling is folded into the PE: out_psum += W_out_tile @ s_t
    every step (single-pass f32r; ~1e-4 relative, well under tolerance).
  - Input matmul base = fused @ W_in runs once in honest fp32.
"""
import numpy as np

import concourse.bacc as bacc
import concourse.mybir as mybir
from concourse.tile import TileContext
from concourse.bass_utils import run_bass_kernel_spmd

B, F, H, O, T = 2048, 768, 1024, 16, 32
NCORES = 8
BC = B // NCORES          # 256 batch rows per core
KH = H // 128             # 8 h-tiles
KF = F // 128             # 6 f-tiles

f32 = mybir.dt.float32
f32r = mybir.dt.float32r

# fp32 arena columns
_WIN_W = KF * 128 * KH        # 6144
_FUSED_W = KF * BC            # 1536
_WOUT_W = KH * O              # 128 (fp32 W_out tiles for the epilogue matmul)
CF = _WIN_W + _FUSED_W + _WOUT_W
# f32r arena columns
_R_W = KH * KH * 128          # 8192
CR = 2 * _R_W                 # 16384
# vec columns: 9 sets of 8 + b_out col
CV = 80


LAST_EXEC_NS = None


def _trunc10(a):
    return (a.view(np.int32) & np.int32(~0x1FFF)).view(np.float32)


def _cols(v):
    """[1024] per-h vector -> [128, 8] (h-tile i in column i)."""
    return np.ascontiguousarray(v.reshape(KH, 128).T)


def _build_program(uniform_df, df_vals):
    nc = bacc.Bacc(None, target_bir_lowering=False)
    d_af = nc.dram_tensor("af", [128, CF], f32, kind="ExternalInput")
    d_ar = nc.dram_tensor("ar", [128, CR], f32, kind="ExternalInput")
    d_vec = nc.dram_tensor("vec", [128, CV], f32, kind="ExternalInput")
    d_out = nc.dram_tensor("out", [O, BC], f32, kind="ExternalOutput")

    AL = mybir.AluOpType

    with TileContext(nc) as tc:
        with (
            tc.tile_pool(name="cpool", bufs=1) as cpool,
            tc.tile_pool(name="spool", bufs=2) as spool,
            tc.tile_pool(name="scr", bufs=8) as scr,
            tc.tile_pool(name="pb", bufs=6, space="PSUM") as pbp,
            tc.tile_pool(name="po", bufs=1, space="PSUM") as pop,
        ):
            af_t = cpool.tile([128, CF], f32, tag="af")
            ar_t = cpool.tile([128, CR], f32r, tag="ar")
            vec_t = cpool.tile([128, CV], f32, tag="vec")
            nc.sync.dma_start(vec_t[:], d_vec[:])
            nc.sync.dma_start(af_t[:], d_af[:])
            nc.sync.dma_start(ar_t[:], d_ar[:].bitcast(f32r))

            def vc(base, i):
                return vec_t[:, base + i:base + i + 1]

            def win(kf, i):
                c = (i * KF + kf) * 128
                return af_t[:, c:c + 128]

            def fusedT(kf):
                c = _WIN_W + kf * BC
                return af_t[:, c:c + BC]

            def rsplit(which, k, i):
                c = which * _R_W + (k * KH + i) * 128
                return ar_t[:, c:c + 128]

            def wof(k):
                c = _WIN_W + _FUSED_W + k * O
                return af_t[:, c:c + O]

            mh = [None] * KH
            na_l = [None] * KH
            s_l = [None] * KH
            th = [None] * KH

            po_t = pop.tile([O, BC], f32, tag="po")
            ssum = [None] * KH
            for i in range(KH):
                ssum[i] = cpool.tile([128, BC], f32, tag=f"ss{i}", name=f"ss{i}")
                nc.gpsimd.memset(ssum[i][:], 0.0)

            HB = KH * BC  # 2048-wide state tiles (uniform path)
            if uniform_df:
                th_b = cpool.tile([128, HB], f32, tag="th_b")
                mh_b = spool.tile([128, HB], f32, tag="mh_b", name="mh_b0")
                na_b = spool.tile([128, HB], f32, tag="na_b", name="na_b0")

            # ---- prologue: base = fused @ W_in (fp32), derive Theta/m^0/na0
            for i in range(KH):
                ps = pbp.tile([128, BC], f32, tag="ps")
                for kf in range(KF):
                    nc.tensor.matmul(ps[:], win(kf, i), fusedT(kf),
                                     start=(kf == 0), stop=(kf == KF - 1))
                if uniform_df:
                    sl = slice(i * BC, (i + 1) * BC)
                    nc.vector.tensor_scalar(th_b[:, sl], ps[:], vc(0, i), vc(8, i),
                                            AL.mult, AL.add)
                    nc.vector.tensor_scalar(mh_b[:, sl], ps[:], vc(16, i), vc(24, i),
                                            AL.mult, AL.add)
                    nc.vector.tensor_scalar(na_b[:, sl], ps[:], vc(32, i), vc(40, i),
                                            AL.mult, AL.add)
                else:
                    th[i] = cpool.tile([128, BC], f32, tag=f"th{i}", name=f"th{i}")
                    nc.vector.tensor_scalar(th[i][:], ps[:], vc(0, i), vc(8, i),
                                            AL.mult, AL.add)
                    mh[i] = spool.tile([128, BC], f32, tag=f"mh{i}", name=f"mh{i}")
                    nc.vector.tensor_scalar(mh[i][:], ps[:], vc(16, i), vc(24, i),
                                            AL.mult, AL.add)
                    na_l[i] = spool.tile([128, BC], f32, tag=f"na{i}", name=f"na{i}")
                    nc.vector.tensor_scalar(na_l[i][:], ps[:], vc(32, i), vc(40, i),
                                            AL.mult, AL.add)

            if uniform_df:
                # ==== wide-op path: scalars are immediates, all 8 h-tiles in
                # one instruction for off-psum ops. s = 1[psum > U], U = Th - t1.
                c_dmf = float(df_vals[0])   # d - f
                c_d = float(df_vals[1])     # d
                c_ndf = float(df_vals[2])   # -d*f
                QW = 2 * BC  # quarter width: 2 h-tiles per t1/u op
                for t in range(T):
                    # t1/u in quarters so each unlocks as its mh slices land
                    t1_b = scr.tile([128, HB], f32, tag="t1b", bufs=1)
                    u_b = scr.tile([128, HB], f32, tag="ub", bufs=1)
                    for qj in range(4):
                        qs = slice(qj * QW, (qj + 1) * QW)
                        nc.vector.scalar_tensor_tensor(
                            t1_b[:, qs], mh_b[:, qs], c_dmf, na_b[:, qs],
                            AL.mult, AL.add)
                        nc.vector.tensor_tensor(u_b[:, qs], th_b[:, qs],
                                                t1_b[:, qs], AL.subtract)
                    mh_new = spool.tile([128, HB], f32, tag="mh_b", name=f"mh_b{t+1}")
                    new_s = [None] * KH
                    for i in range(KH):
                        sl = slice(i * BC, (i + 1) * BC)
                        ps = None
                        if t > 0:
                            ps = pbp.tile([128, BC], f32, tag="ps")
                            mm = 0
                            for k in range(KH):
                                for which in (0, 1):
                                    nc.tensor.matmul(ps[:], rsplit(which, k, i),
                                                     s_l[k][:],
                                                     start=(mm == 0),
                                                     stop=(mm == 2 * KH - 1))
                                    mm += 1
                        s_new = spool.tile([128, BC], f32r, tag=f"s{i}",
                                           name=f"s{i}_{t}")
                        if t > 0:
                            nc.vector.tensor_tensor(s_new[:], ps[:], u_b[:, sl],
                                                    AL.is_gt)
                            Mt = scr.tile([128, BC], f32, tag="M", bufs=4)
                            nc.vector.tensor_tensor(Mt[:], ps[:], t1_b[:, sl],
                                                    AL.add)
                        else:
                            # no rec at t=0: psum == 0, so s = 1[0 > U], M = t1
                            nc.vector.tensor_scalar(s_new[:], u_b[:, sl], 0.0, None,
                                                    AL.is_lt)
                            Mt = None
                        src_m = Mt[:] if Mt is not None else t1_b[:, sl]
                        nc.gpsimd.tensor_tensor(mh_new[:, sl], src_m,
                                                s_new[:].bitcast(f32), AL.subtract)
                        new_s[i] = s_new
                    # na update + spike-count accumulation off the critical path
                    t2_b = scr.tile([128, HB], f32, tag="t2b", bufs=1)
                    nc.scalar.activation(t2_b[:], mh_b[:],
                                         mybir.ActivationFunctionType.Copy,
                                         bias=0.0, scale=c_ndf)
                    na_new = spool.tile([128, HB], f32, tag="na_b", name=f"na_b{t+1}")
                    nc.vector.scalar_tensor_tensor(
                        na_new[:], na_b[:], c_d, t2_b[:], AL.mult, AL.add)
                    for i in range(KH):
                        nc.gpsimd.tensor_tensor(ssum[i][:], ssum[i][:],
                                                new_s[i][:].bitcast(f32), AL.add)
                    s_l = new_s
                    mh_b = mh_new
                    na_b = na_new
            else:
                # ==== general per-tile path (per-partition scalar APs)
                for t in range(T):
                    new_s = [None] * KH
                    new_mh = [None] * KH
                    new_na = [None] * KH
                    for i in range(KH):
                        ps = None
                        if t > 0:
                            ps = pbp.tile([128, BC], f32, tag="ps")
                            mm = 0
                            for k in range(KH):
                                for which in (0, 1):
                                    nc.tensor.matmul(ps[:], rsplit(which, k, i),
                                                     s_l[k][:],
                                                     start=(mm == 0),
                                                     stop=(mm == 2 * KH - 1))
                                    mm += 1
                        t1 = scr.tile([128, BC], f32, tag="t1")
                        nc.vector.scalar_tensor_tensor(
                            t1[:], mh[i][:], vc(48, i), na_l[i][:],
                            AL.mult, AL.add)
                        u_t = scr.tile([128, BC], f32, tag="u")
                        nc.vector.tensor_tensor(u_t[:], th[i][:], t1[:],
                                                AL.subtract)
                        s_new = spool.tile([128, BC], f32r, tag=f"s{i}",
                                           name=f"s{i}_{t}")
                        if t > 0:
                            nc.vector.tensor_tensor(s_new[:], ps[:], u_t[:],
                                                    AL.is_gt)
                            Mt = scr.tile([128, BC], f32, tag="M")
                            nc.vector.tensor_tensor(Mt[:], ps[:], t1[:], AL.add)
                        else:
                            nc.vector.tensor_scalar(s_new[:], u_t[:], 0.0, None,
                                                    AL.is_lt)
                            Mt = t1
                        mh_new = spool.tile([128, BC], f32, tag=f"mh{i}",
                                            name=f"mh{i}_{t}")
                        nc.gpsimd.tensor_tensor(mh_new[:], Mt[:],
                                                s_new[:].bitcast(f32), AL.subtract)
                        t2 = scr.tile([128, BC], f32, tag="t2")
                        nc.scalar.activation(t2[:], mh[i][:],
                                             mybir.ActivationFunctionType.Copy,
                                             bias=0.0, scale=vc(64, i))
                        na_new = spool.tile([128, BC], f32, tag=f"na{i}",
                                            name=f"na{i}_{t}")
                        nc.vector.scalar_tensor_tensor(
                            na_new[:], na_l[i][:], vc(56, i), t2[:],
                            AL.mult, AL.add)
                        nc.gpsimd.tensor_tensor(ssum[i][:], ssum[i][:],
                                                s_new[:].bitcast(f32), AL.add)
                        new_s[i] = s_new
                        new_mh[i] = mh_new
                        new_na[i] = na_new
                    s_l, mh, na_l = new_s, new_mh, new_na

            # ---- epilogue: out = (ssum @ W_out)/T + b_out (honest fp32)
            for k in range(KH):
                nc.tensor.matmul(po_t[:], wof(k), ssum[k][:],
                                 start=(k == 0), stop=(k == KH - 1))
            outt = cpool.tile([O, BC], f32, tag="outt")
            nc.vector.tensor_scalar(outt[:], po_t[:], 1.0 / T,
                                    vec_t[0:O, 72:73], AL.mult, AL.add)
            nc.sync.dma_start(d_out[:], outt[:])

    nc.compile()
    return nc


def kernel(fused, W_in, b_in, W_rec, b_rec, W_out, b_out,
           resonant_decay, resonant_frequency):
    fused = np.ascontiguousarray(fused, np.float32)
    W_in = np.ascontiguousarray(W_in, np.float32)
    W_rec = np.ascontiguousarray(W_rec, np.float32)
    W_out = np.ascontiguousarray(W_out, np.float32)
    b_in = np.asarray(b_in, np.float32)
    b_rec = np.asarray(b_rec, np.float32)
    b_out = np.asarray(b_out, np.float32)
    rd = np.asarray(resonant_decay, np.float32)
    rf = np.asarray(resonant_frequency, np.float32)

    one = np.float32(1.0)
    d = (np.float32(0.55) + np.float32(0.4) / (one + np.exp(-rd))).astype(np.float32)
    f = (np.float32(0.1) + np.float32(0.9) / (one + np.exp(-rf))).astype(np.float32)
    d64, f64 = d.astype(np.float64), f.astype(np.float64)
    den = (1.0 - d64) ** 2 + f64
    c1 = ((1.0 - d64) / den).astype(np.float32)
    c2 = (d64 * f64 / den).astype(np.float32)
    bb = (b_in + b_rec).astype(np.float32)

    vec_np = np.zeros((128, CV), np.float32)
    vec_np[:, 0:8] = _cols(-c1)
    vec_np[:, 8:16] = _cols(one - c1 * bb)
    vec_np[:, 16:24] = _cols(-c1)
    vec_np[:, 24:32] = _cols(-(c1 * bb))
    vec_np[:, 32:40] = _cols(c2)
    vec_np[:, 40:48] = _cols(c2 * bb)
    vec_np[:, 48:56] = _cols(d - f)
    vec_np[:, 56:64] = _cols(d)
    vec_np[:, 64:72] = _cols(-(d64 * f64).astype(np.float32))
    vec_np[0:O, 72] = b_out

    uniform_df = (np.ptp(d) == 0.0) and (np.ptp(f) == 0.0)
    df_vals = (float(d[0] - f[0]), float(d[0]),
               float(-(d64[0] * f64[0]))) if uniform_df else (0.0, 0.0, 0.0)

    # f32r arena (same for all cores): W_rec residual split + W_out tiles
    R1 = _trunc10(W_rec)
    R2 = (W_rec - R1).astype(np.float32)
    ar_np = np.empty((128, CR), np.float32)
    for k in range(KH):
        for i in range(KH):
            ar_np[:, (k * KH + i) * 128:(k * KH + i + 1) * 128] = \
                R1[k * 128:(k + 1) * 128, i * 128:(i + 1) * 128]
            ar_np[:, _R_W + (k * KH + i) * 128:_R_W + (k * KH + i + 1) * 128] = \
                R2[k * 128:(k + 1) * 128, i * 128:(i + 1) * 128]

    # fp32 arena: W_in tiles (shared) + per-core transposed fused shard
    af_shared = np.empty((128, _WIN_W), np.float32)
    for i in range(KH):
        for kf in range(KF):
            af_shared[:, (i * KF + kf) * 128:(i * KF + kf + 1) * 128] = \
                W_in[kf * 128:(kf + 1) * 128, i * 128:(i + 1) * 128]

    nc = _build_program(uniform_df, df_vals)

    in_maps = []
    for c in range(NCORES):
        af_np = np.empty((128, CF), np.float32)
        af_np[:, :_WIN_W] = af_shared
        shard = fused[c * BC:(c + 1) * BC, :]          # [256, 768]
        shardT = np.ascontiguousarray(shard.T)         # [768, 256]
        for kf in range(KF):
            af_np[:, _WIN_W + kf * BC:_WIN_W + (kf + 1) * BC] = \
                shardT[kf * 128:(kf + 1) * 128, :]
        for k in range(KH):
            af_np[:, _WIN_W + _FUSED_W + k * O:_WIN_W + _FUSED_W + (k + 1) * O] = \
                W_out[k * 128:(k + 1) * 128, :]
        in_maps.append({"af": af_np, "ar": ar_np, "vec": vec_np})

    res = run_bass_kernel_spmd(nc, in_maps, core_ids=list(range(NCORES)))
    global LAST_EXEC_NS
    LAST_EXEC_NS = res.exec_time_ns

    out_full = np.empty((B, O), np.float32)
    for c in range(NCORES):
        out_full[c * BC:(c + 1) * BC, :] = res.results[c]["out"].T
    return out_full
